# revision 1
# baseline (speedup 1.0000x reference)
"""Trainium2 Bass kernel for nn_CapsuleLayer (dynamic routing), v2.

Reference computation (B=128, I=1152, P=8, J=10, D=16):
    inputs_hat[b,i,j,d] = sum_p W[i,j,d,p] * inputs[b,i,p]
    b_logits = 0
    3x routing:
        c = softmax_j(b_logits)
        s[b,j,d] = sum_i c[b,i,j] * inputs_hat[b,i,j,d]
        outputs = squash(s)
        b_logits += sum_d inputs_hat[b,i,j,d] * outputs[b,j,d]   (iters 0,1)

v2 never materializes inputs_hat.  Both routing contractions run on the
PE by regrouping the triple products W*x*c and W*x*o:

  weighted sum:  s_j^T[d,b] = sum_{k=(p,i)} W2P[k,(d,j)] * zT_j[k,b]
                 with z_j = c_j (*) x  (small DVE mul, c broadcast over
                 p stays 2x-packed in (i_hi, p, i_lo) layout), zT via PE
                 transposes (bf16 PSUM -> 2x drains)
  agreement:     A_j[b,(p,i)] = sum_d o[b,(d,j)] * W[i,j,d,p]  (PE;
                 o^T stationary at 32-row slots, j-slot pads of the
                 moving W operand are zero), then
                 agr_j = sum_p x (*) A_j  (DVE mul + 3-level p-tree)

i-sharded across 8 cores (IL=144 each), batch B=128 on partitions.
Iteration-0 s comes from a dense K=(i,p) matmul (c==1/J); its AllReduce
overlaps the one-time W2P/WD4 builds.  The two mid-loop AllReduces are
bf16 and split into j-halves so each half's collective pipelines against
the other half's weighted-sum / agreement work.  squash's rsqrt runs
entirely on DVE (quake seed + 2 Newton steps) so ACT's function table
stays on Exp.
"""

import os
import sys
import functools

import numpy as np

if "/opt/trn_rl_repo" not in sys.path:
    sys.path.insert(0, "/opt/trn_rl_repo")

B = 128
I_FULL = 1152
P_DIM = 8
J = 10
D = 16
JD = D * J  # 160, flattened (d, j): col = d*J + j
NCORES = 8
ROUTINGS = 3
EPS = 1e-7

MM_DT = os.environ.get("K_MM_DT", "f32r")
DEBUG_TAP = os.environ.get("K_DEBUG", "")
CC_BF16 = os.environ.get("K_CC_BF16", "1") == "1"
CC_ENG = os.environ.get("K_CC_ENG", "sync")
SPLIT_AR = os.environ.get("K_SPLIT_AR", "1") == "1"
# halves for the split-AR pipeline: (X-groups, j0, nj)
JHALVES = [([0, 1], 0, 6), ([2, 3], 6, 4)]

# j <-> (group X, slot q): PE out column positions are limited to
# {0, 32, 64} (quadrant 3 unusable), so 3 slots per PSUM tile
JGROUPS = [(0, 3), (3, 3), (6, 3), (9, 1)]


def build(n_cores, IL, repeat=1):
    """Trace + compile the SPMD Bass program (one program, all cores)."""
    import concourse.bacc as bacc
    import concourse.bass as bass
    import concourse.mybir as mybir
    import concourse.tile as tile
    from concourse.masks import make_identity

    F32 = mybir.dt.float32
    BF16 = mybir.dt.bfloat16
    AF = mybir.ActivationFunctionType
    OP = mybir.AluOpType
    AX = mybir.AxisListType

    assert IL % 16 == 0
    G = IL // 16  # number of 128-row (16 i x 8 p) k-tiles

    nc = bacc.Bacc(
        "TRN2", target_bir_lowering=False, debug=False, num_devices=n_cores
    )
    x_d = nc.dram_tensor("x", [B, IL, P_DIM], F32, kind="ExternalInput").ap()
    w_d = nc.dram_tensor("w", [IL, J, D, P_DIM], F32, kind="ExternalInput").ap()
    out_d = nc.dram_tensor("out", [B, J, D], F32, kind="ExternalOutput").ap()

    with tile.TileContext(nc, num_cores=n_cores) as tc:
        for rep in range(repeat):
            _trace(tc, nc, x_d, w_d, out_d, n_cores, IL, G, F32, BF16, AF,
                   OP, AX, bass, mybir, make_identity, rep)

    nc.compile()
    return nc


def _trace(tc, nc, x_d, w_d, out_d, n_cores, IL, G, F32, BF16, AF, OP, AX,
           bass, mybir, make_identity, rep=0):
    import contextlib

    F32R = mybir.dt.float32r
    OPDT = {"f32": F32, "f32r": F32R, "bf16": BF16}[MM_DT]
    CCDT = BF16 if CC_BF16 else F32

    ctx = contextlib.ExitStack()
    with ctx:
        singles = ctx.enter_context(
            tc.tile_pool(name=f"singles{rep}", bufs=1))
        stage = ctx.enter_context(tc.tile_pool(name=f"stage{rep}", bufs=8))
        big = ctx.enter_context(tc.tile_pool(name=f"big{rep}", bufs=1))
        small = ctx.enter_context(tc.tile_pool(name=f"small{rep}", bufs=3))
        dram = ctx.enter_context(
            tc.tile_pool(name=f"dram{rep}", bufs=1, space="DRAM"))

        # ---- constants -------------------------------------------------
        ident = singles.tile([128, 128], F32)
        make_identity(nc, ident[:])
        ident_b16 = singles.tile([128, 128], BF16)
        nc.vector.tensor_copy(ident_b16[:], ident[:])
        dummy = singles.tile([128, 1], F32)
        nc.vector.memset(dummy[:], 0.0)
        U32 = mybir.dt.uint32
        # 0x5f3759df seed row for the DVE-only rsqrt (keeps ACT's table on Exp)
        qseed = singles.tile([128, J], U32)
        nc.vector.memset(qseed[:].bitcast(F32), 0.0)
        nc.vector.tensor_scalar_add(qseed[:], qseed[:], 0x5f3759df)
        eps_t = singles.tile([128, 1], F32)
        nc.vector.memset(eps_t[:], EPS)
        nc.scalar.activation(dummy[:], dummy[:], AF.Exp)

        # permutation P[k, m] = 1 iff m = (k%8)*16 + k//8 (row (i_lo,p) ->
        # (p,i_lo)); solutions of k - 8m + 127p == 0 are exactly that line
        perm = singles.tile([128, 128], BF16)
        nc.gpsimd.memset(perm[:], 0.0)
        for p in range(P_DIM):
            nc.gpsimd.affine_select(
                out=perm[:], in_=perm[:],
                compare_op=mybir.AluOpType.not_equal,
                fill=1.0, base=127 * p,
                pattern=[[-8, 128]], channel_multiplier=1,
            )

        dma_engs = [nc.sync, nc.scalar]
        cc_eng = {"sync": nc.sync, "gpsimd": nc.gpsimd,
                  "scalar": nc.scalar}[CC_ENG]

        # ---- load x ----------------------------------------------------
        x_nat = big.tile([128, IL * P_DIM], F32)
        nc.sync.dma_start(out=x_nat[:], in_=x_d.rearrange("b i p -> b (i p)"))

        # ---- setup phase (own PSUM pools, closed before routing) -------
        setup_ctx = contextlib.ExitStack()
        psT = setup_ctx.enter_context(
            tc.tile_pool(name=f"psT{rep}", bufs=2, space="PSUM"))
        psW = setup_ctx.enter_context(
            tc.tile_pool(name=f"psW{rep}", bufs=2, space="PSUM"))
        psS0 = setup_ctx.enter_context(
            tc.tile_pool(name=f"psS0{rep}", bufs=1, space="PSUM"))

        xT = big.tile([128, G, 128], OPDT)
        for g in range(G):
            pst = psT.tile([128, 128], F32, tag="pst")
            nc.tensor.transpose(
                pst[:], x_nat[:, g * 128:(g + 1) * 128], ident[:])
            nc.vector.tensor_copy(xT[:, g, :], pst[:])

        # stage W2[k=(i_lo,p), g, (d,j)]; accumulate s0 as each g lands
        W2 = big.tile([128, G, JD], OPDT)
        ps_a = psS0.tile([128, 128], F32, tag="s0a")
        ps_b = psS0.tile([32, 128], F32, tag="s0b")

        stage_engs = [nc.sync, nc.scalar, nc.gpsimd]
        wna2s = {}
        for g0 in range(0, G, 2):
            ng = min(2, G - g0)
            wna2 = stage.tile([128, ng * 16, P_DIM], F32, tag="wna")
            stage_engs[(g0 // 2) % 3].dma_start(
                out=wna2[:],
                in_=w_d[16 * g0:16 * (g0 + ng), 0:8, :, :].rearrange(
                    "i j d p -> (j d) i p"),
            )
            wna2s[g0] = wna2
        wnb4s = {}
        for g0 in range(0, G, 4):
            ng = min(4, G - g0)
            wnb4 = stage.tile([32, ng * 16, P_DIM], F32, tag="wnb")
            stage_engs[2 - (g0 // 4) % 3].dma_start(
                out=wnb4[:],
                in_=w_d[16 * g0:16 * (g0 + ng), 8:10, :, :].rearrange(
                    "i j d p -> (j d) i p"),
            )
            wnb4s[g0] = wnb4

        for g in range(G):
            wna = wna2s[g - g % 2][:, (g % 2) * 16:(g % 2) * 16 + 16, :]
            wnb = wnb4s[g - g % 4][:, (g % 4) * 16:(g % 4) * 16 + 16, :]
            W2g = W2[:, g, :].rearrange("k (d j) -> k d j", d=D, j=J)
            psa = psT.tile([128, 128], F32, tag="pst")
            nc.tensor.transpose(psa[:], wna, ident[:])
            nc.vector.tensor_copy(
                W2g[:, :, 0:8],
                psa[:].rearrange("k (j d) -> k j d", j=8, d=D).transpose(
                    [0, 2, 1]))
            psb = psT.tile([128, 128], F32, tag="pst")
            nc.tensor.transpose(psb[:, 0:32], wnb, ident[0:32, 0:32])
            nc.vector.tensor_copy(
                W2g[:, :, 8:10],
                psb[:, 0:32].rearrange("k (j d) -> k j d", j=2, d=D)
                    .transpose([0, 2, 1]))
            nc.tensor.matmul(ps_a[:], W2[:, g, 0:128], xT[:, g, :],
                             start=(g == 0), stop=(g == G - 1))
            nc.tensor.matmul(ps_b[:], W2[:, g, 128:JD], xT[:, g, :],
                             start=(g == 0), stop=(g == G - 1))

        # s0 drain: scale by 1/J, transpose back to [b, (d,j)]
        s0T_a = stage.tile([128, 128], F32, tag="s0Ta")
        s0T_b = stage.tile([32, 128], F32, tag="s0Tb")
        nc.scalar.mul(s0T_a[:], ps_a[:], 1.0 / J)
        nc.scalar.mul(s0T_b[:], ps_b[:], 1.0 / J)
        s0p = small.tile([128, JD], CCDT, tag="spart")
        pst0 = psT.tile([128, 128], F32, tag="pst")
        nc.tensor.transpose(pst0[:], s0T_a[:], ident[:])
        nc.vector.tensor_copy(s0p[:, 0:128], pst0[:])
        pst0b = psT.tile([128, 128], F32, tag="pst")
        nc.tensor.transpose(pst0b[:, 0:32], s0T_b[:], ident[0:32, 0:32])
        nc.vector.tensor_copy(s0p[:, 128:JD], pst0b[:, 0:32])

        def all_reduce(s_part, tag, width=JD):
            # s_part dtype must match CCDT
            cc_in = dram.tile([B, width], CCDT, name=f"ccin_{tag}_{rep}")
            cc_out = dram.tile([B, width], CCDT, name=f"ccout_{tag}_{rep}",
                               addr_space="Shared")
            cc_eng.dma_start(out=cc_in[:], in_=s_part[:])
            if n_cores > 1 and os.environ.get("K_NO_CC", "0") != "1":
                nc.gpsimd.collective_compute(
                    "AllReduce",
                    OP.add,
                    replica_groups=[list(range(n_cores))],
                    ins=[cc_in[:].opt()],
                    outs=[cc_out[:].opt()],
                )
            else:
                cc_eng.dma_start(out=cc_out[:], in_=cc_in[:])
            s_glob = small.tile([128, width], CCDT, tag=f"sglob{width}")
            cc_eng.dma_start(out=s_glob[:], in_=cc_out[:])
            return s_glob

        s0g = all_reduce(s0p, "s0")  # overlaps the builds below

        # ---- one-time builds (hidden under AR(s0)) ---------------------
        # xP[b, (i_hi, p, i_lo)] bf16: p pulled inside so the per-j c-mul
        # keeps a packed 16-wide innermost dim
        xP = big.tile([128, G, P_DIM, 16], BF16)
        nc.vector.tensor_copy(
            xP[:],
            x_nat[:].rearrange("b (ih il p) -> b ih p il", ih=G, il=16,
                               p=P_DIM))
        W2b16 = big.tile([128, G, JD], BF16)
        nc.vector.tensor_copy(W2b16[:], W2[:])
        W2P = big.tile([128, G, JD], BF16)
        for g in range(G):
            psp = psT.tile([128, 128], F32, tag="pst")
            nc.tensor.matmul(psp[:], perm[:], W2b16[:, g, 0:128],
                             start=True, stop=True)
            nc.vector.tensor_copy(W2P[:, g, 0:128], psp[:])
            psp2 = psT.tile([128, 128], F32, tag="pst")
            nc.tensor.matmul(psp2[:, 0:32], perm[:], W2b16[:, g, 128:JD],
                             start=True, stop=True)
            nc.vector.tensor_copy(W2P[:, g, 128:JD], psp2[:, 0:32])

        # WD4[32q+d, g, (p,i_lo)] = W[i,j(X,q),d,p]: per-(g,j) transposes
        # of W2P's d-columns into 32-row slots; pad rows stay zero because
        # the two ring PSUM buffers are zeroed once and the transposes
        # only ever write rows 32q..32q+16
        WD4 = [big.tile([128, G, 128], BF16, name=f"wd4_{xi}_{rep}")
               for xi in range(len(JGROUPS))]
        W2Pv = W2P[:].rearrange("k g (d j) -> k g d j", d=D, j=J)
        for xi, (j0, nj) in enumerate(JGROUPS):
            for g in range(G):
                psw = psW.tile([128, 128], BF16, tag="psw")
                nc.vector.memset(psw[:].bitcast(F32), 0.0)
                for q in range(nj):
                    nc.tensor.transpose(
                        psw[32 * q:32 * q + 16, :],
                        W2Pv[:, g, :, j0 + q], ident_b16[:])
                nc.vector.tensor_copy(WD4[xi][:, g, :], psw[:])

        setup_ctx.close()

        # ---- routing PSUM pools (created after setup pools released) ---
        psO = ctx.enter_context(
            tc.tile_pool(name=f"psO{rep}", bufs=1, space="PSUM"))
        psA = ctx.enter_context(
            tc.tile_pool(name=f"psA{rep}", bufs=2, space="PSUM"))
        psZ = ctx.enter_context(
            tc.tile_pool(name=f"psZ{rep}", bufs=2, space="PSUM"))
        psS = ctx.enter_context(
            tc.tile_pool(name=f"psS{rep}", bufs=1, space="PSUM"))

        # ---- routing buffers -------------------------------------------
        L = big.tile([128, IL, J], F32)
        CbT = big.tile([128, J, IL], BF16)
        Asbs = [big.tile([128, G, P_DIM, 16], BF16, name=f"asb{k}_{rep}")
                for k in range(2)]
        Zall = big.tile([128, J, G, P_DIM, 16], BF16)
        zTs = [big.tile([128, G, 128], BF16, name=f"zt{k}_{rep}")
               for k in range(2)]
        sT = big.tile([128, len(JGROUPS), 128], F32)
        oT4s = [big.tile([128, 128], BF16, name=f"ot4_{xi}_{rep}")
                for xi in range(len(JGROUPS))]
        for t in oT4s:
            nc.vector.memset(t[:], 0.0)
        Lv = L[:].rearrange("b (ih il) j -> b ih il j", ih=G, il=16)
        W2Pj = W2P[:].rearrange("k g (d j) -> k g d j", d=D, j=J)

        def squash(s_glob, want_bf16, nj=J):
            """squash along d of s_glob[128,(d j)] -> (f32, bf16|None)."""
            sq = small.tile([128, D * nj], F32, tag=f"sq{nj}")
            nc.vector.tensor_mul(sq[:], s_glob[:], s_glob[:])
            s2 = small.tile([128, nj], F32, tag=f"s2{nj}")
            nc.vector.reduce_sum(
                s2[:], sq.rearrange("b (d j) -> b j d", d=D, j=nj), axis=AX.X)
            # rt = rsqrt(s2 + eps) entirely on DVE (quake seed + 2 Newton
            # steps) so ACT never swaps its function table off Exp
            v = small.tile([128, nj], F32, tag=f"v{nj}")
            nc.vector.tensor_scalar_add(v[:], s2[:], EPS)
            rt = small.tile([128, nj], F32, tag=f"rt{nj}")
            nc.vector.tensor_scalar(
                rt[:].bitcast(U32), v[:].bitcast(U32), 1, None,
                op0=OP.logical_shift_right)
            nc.vector.tensor_tensor(
                rt[:].bitcast(U32), qseed[:, 0:nj], rt[:].bitcast(U32),
                op=OP.subtract)
            t1 = small.tile([128, nj], F32, tag=f"t1{nj}")
            for _ in range(2):
                nc.vector.tensor_mul(t1[:], rt[:], rt[:])
                nc.vector.tensor_mul(t1[:], t1[:], v[:])
                nc.vector.tensor_scalar(
                    t1[:], t1[:], -0.5, 1.5, op0=OP.mult, op1=OP.add)
                nc.vector.tensor_mul(rt[:], rt[:], t1[:])
            u = small.tile([128, nj], F32, tag=f"u{nj}")
            nc.vector.tensor_scalar_add(u[:], s2[:], 1.0)
            ru = small.tile([128, nj], F32, tag=f"ru{nj}")
            nc.vector.reciprocal(ru[:], u[:])
            sc = small.tile([128, nj], F32, tag=f"sc{nj}")
            nc.vector.tensor_mul(sc[:], s2[:], ru[:])
            nc.vector.tensor_mul(sc[:], sc[:], rt[:])
            o_f = small.tile([128, D * nj], F32, tag=f"of{nj}")
            sc_b = sc[:].unsqueeze(1).broadcast_to([128, D, nj])
            nc.vector.tensor_tensor(
                o_f.rearrange("b (d j) -> b d j", d=D, j=nj),
                s_glob.rearrange("b (d j) -> b d j", d=D, j=nj),
                sc_b, op=OP.mult)
            o_b = None
            if want_bf16:
                o_b = small.tile([128, D * nj], BF16, tag=f"ob{nj}")
                nc.vector.tensor_copy(o_b[:], o_f[:])
            return o_f, o_b

        def agreement(o_b, first, groups=None, oj0=0, onj=J):
            """L (+)= sum_d IH*o via PE: A_j = oT_j @ WD_j, then
            agr_j = sum_p xP (*) A_j (DVE mul + p-tree).  o_b holds
            j-columns oj0..oj0+onj; groups selects X-groups."""
            if groups is None:
                groups = list(range(len(JGROUPS)))
            o_bv = o_b[:].rearrange("b (d j) -> b d j", d=D, j=onj)
            for xi in groups:
                j0, nj = JGROUPS[xi]
                pso = psO.tile([128, 128], BF16, tag="pso")
                for q in range(nj):
                    nc.tensor.transpose(
                        pso[32 * q:32 * q + 16, :],
                        o_bv[:, :, j0 + q - oj0], ident_b16[:])
                for q in range(nj):
                    nc.vector.tensor_copy(
                        oT4s[xi][32 * q:32 * q + 16, :],
                        pso[32 * q:32 * q + 16, :])
            for xi in groups:
                j0, nj = JGROUPS[xi]
                for q in range(nj):
                    j = j0 + q
                    Asb = Asbs[j % 2]
                    Af = Asb[:].rearrange("b g p il -> b (g p il)")
                    for c0 in range(0, G, 4):
                        ngc = min(4, G - c0)
                        psa_t = psA.tile([128, 512], F32, tag="psa")
                        for g in range(c0, c0 + ngc):
                            nc.tensor.matmul(
                                psa_t[:, 128 * (g - c0):128 * (g - c0 + 1)],
                                oT4s[xi][32 * q:32 * q + 32, :],
                                WD4[xi][32 * q:32 * q + 32, g, :],
                                start=True, stop=True,
                                tile_position=(32 * q, 0))
                        nc.scalar.copy(
                            Af[:, 128 * c0:128 * (c0 + ngc)],
                            psa_t[:, 0:128 * ngc])
                    nc.vector.tensor_tensor(Asb[:], xP[:], Asb[:],
                                            op=OP.mult)
                    w = P_DIM
                    while w > 1:
                        h = w // 2
                        nc.vector.tensor_tensor(
                            Asb[:, :, 0:h, :], Asb[:, :, 0:h, :],
                            Asb[:, :, h:w, :], op=OP.add)
                        w = h
                    if first:
                        nc.vector.tensor_copy(
                            Lv[:, :, :, j], Asb[:, :, 0, :])
                    else:
                        nc.vector.tensor_tensor(
                            Lv[:, :, :, j], Lv[:, :, :, j],
                            Asb[:, :, 0, :], op=OP.add)

        def softmax():
            """CbT[b, j, i] = softmax_j(L)."""
            E = big.tile([128, IL, J], F32, tag="E")
            nc.scalar.activation(E[:], L[:], AF.Exp)
            Z = small.tile([128, IL], F32, tag="Z")
            nc.vector.reduce_sum(Z[:], E[:], axis=AX.X)
            R = small.tile([128, IL], F32, tag="R")
            nc.vector.reciprocal(R[:], Z[:])
            nc.vector.tensor_tensor(
                CbT[:].rearrange("b j i -> b i j"),
                E[:], R[:].unsqueeze(2).broadcast_to([128, IL, J]),
                op=OP.mult)

        def ws_matmuls(psS_t, groups):
            for xi in groups:
                j0, nj = JGROUPS[xi]
                for q in range(nj):
                    j = j0 + q
                    zT = zTs[j % 2]
                    cb = CbT[:, j, :].rearrange(
                        "b (ih il) -> b ih il", ih=G).unsqueeze(2)
                    nc.vector.tensor_tensor(
                        Zall[:, j], xP[:],
                        cb.broadcast_to([128, G, P_DIM, 16]), op=OP.mult)
                    zjf = Zall[:, j].rearrange("b g p il -> b (g p il)")
                    zTf = zT[:].rearrange("k g b -> k (g b)")
                    for t0 in range(0, G, 4):
                        ngt = min(4, G - t0)
                        psz = psZ.tile([128, 512], BF16, tag="psz")
                        for t in range(t0, t0 + ngt):
                            nc.tensor.transpose(
                                psz[:, 128 * (t - t0):128 * (t - t0 + 1)],
                                zjf[:, 128 * t:128 * (t + 1)],
                                ident_b16[:])
                        nc.vector.tensor_copy(
                            zTf[:, 128 * t0:128 * (t0 + ngt)],
                            psz[:, 0:128 * ngt])
                    for t in range(G):
                        nc.tensor.matmul(
                            psS_t[32 * q:32 * q + 16,
                                  128 * xi:128 * xi + 128],
                            W2Pj[:, t, :, j], zT[:, t, :],
                            start=(t == 0), stop=(t == G - 1))
        def ws_drain_groups(psS_t, groups, j0h, njh):
            x0, x1 = groups[0], groups[-1] + 1
            nc.scalar.copy(
                sT[:, x0:x1, :].rearrange("b x k -> b (x k)"),
                psS_t[:, 128 * x0:128 * x1])
            s_part = small.tile([128, D * njh], CCDT, tag=f"spart{njh}")
            spv = s_part.rearrange("b (d j) -> b d j", d=D, j=njh)
            for xi in groups:
                j0, nj = JGROUPS[xi]
                psb_t = psO.tile([128, 128], F32, tag="psb")
                nc.tensor.transpose(psb_t[:], sT[:, xi, :], ident[:])
                nc.vector.tensor_copy(
                    spv[:, :, j0 - j0h:j0 - j0h + nj],
                    psb_t[:].rearrange("b (sq d) -> b sq d", sq=4, d=32)
                         [:, 0:nj, 0:16].transpose([0, 2, 1]))
            return s_part

        def weighted_sum():
            psS_t = psS.tile([128, 512], F32, tag="pss")
            ws_matmuls(psS_t, list(range(len(JGROUPS))))
            return ws_drain_groups(
                psS_t, list(range(len(JGROUPS))), 0, J)

        def weighted_sum_split():
            psS_t = psS.tile([128, 512], F32, tag="pss")
            parts = []
            for groups, j0h, njh in JHALVES:
                ws_matmuls(psS_t, groups)
                parts.append(ws_drain_groups(psS_t, groups, j0h, njh))
            return parts

        # ---- routing ----------------------------------------------------
        def emit(src_dj):
            OUTJD = small.tile([128, J, D], F32, tag="outjd")
            nc.vector.tensor_copy(
                OUTJD[:], src_dj.rearrange("b (d j) -> b j d", d=D, j=J))
            nc.sync.dma_start(out=out_d[:], in_=OUTJD[:])

        _, ob0 = squash(s0g, want_bf16=True)
        if DEBUG_TAP == "s0":
            return emit(s0g[:])
        agreement(ob0, first=True)
        if DEBUG_TAP == "L0":
            OUTJD = small.tile([128, J, D], F32, tag="outjd")
            nc.vector.tensor_copy(
                OUTJD[:],
                L[:, 0:16, :].rearrange("b i j -> b j i"))
            nc.sync.dma_start(out=out_d[:], in_=OUTJD[:])
            return
        softmax()
        if not SPLIT_AR:
            s1p = weighted_sum()
            if DEBUG_TAP == "s1p":
                return emit(s1p[:])
            s1g = all_reduce(s1p, "s1")
            _, ob1 = squash(s1g, want_bf16=True)
            agreement(ob1, first=False)
            softmax()
            s2p = weighted_sum()
            s2g = all_reduce(s2p, "s2")
            o_f, _ = squash(s2g, want_bf16=False)
            emit(o_f[:])
        else:
            parts = weighted_sum_split()
            globs = [all_reduce(p, f"s1h{hi}", width=D * JHALVES[hi][2])
                     for hi, p in enumerate(parts)]
            for hi, (groups, j0h, njh) in enumerate(JHALVES):
                _, obh = squash(globs[hi], want_bf16=True, nj=njh)
                agreement(obh, first=False, groups=groups,
                          oj0=j0h, onj=njh)
            softmax()
            parts = weighted_sum_split()
            globs = [all_reduce(p, f"s2h{hi}", width=D * JHALVES[hi][2])
                     for hi, p in enumerate(parts)]
            OUTJD = small.tile([128, J, D], F32, tag="outjd")
            for hi, (groups, j0h, njh) in enumerate(JHALVES):
                o_fh, _ = squash(globs[hi], want_bf16=False, nj=njh)
                nc.vector.tensor_copy(
                    OUTJD[:, j0h:j0h + njh, :],
                    o_fh.rearrange("b (d j) -> b j d", d=D, j=njh))
            nc.sync.dma_start(out=out_d[:], in_=OUTJD[:])


@functools.lru_cache(maxsize=None)
def _get_nc():
    return build(NCORES, I_FULL // NCORES)


def kernel(inputs, W):
    """Full-input entry point: inputs [128,1152,8] f32, W [1,1152,10,16,8]."""
    from concourse.bass_utils import run_bass_kernel_spmd

    inputs = np.ascontiguousarray(np.asarray(inputs), dtype=np.float32)
    W0 = np.ascontiguousarray(np.asarray(W)[0], dtype=np.float32)
    IL = I_FULL // NCORES
    nc = _get_nc()
    in_maps = [
        {
            "x": np.ascontiguousarray(inputs[:, c * IL:(c + 1) * IL, :]),
            "w": np.ascontiguousarray(W0[c * IL:(c + 1) * IL]),
        }
        for c in range(NCORES)
    ]
    res = run_bass_kernel_spmd(nc, in_maps, core_ids=list(range(NCORES)))
    return np.asarray(res.results[0]["out"], dtype=np.float32)


if __name__ == "__main__":
    nc = build(1, 16)
    print("built OK")

